# revision 62
# baseline (speedup 1.0000x reference)
"""DiagonalSSMLayer Trainium2 kernel, v6.

Full (unsharded) inputs in, full output out. Data-parallel over batch across
8 NeuronCores (B=8, one batch element per core). Host casts x to fp16
(rel-err budget 2e-2 >> fp16's 5e-4) and also ships a pre-transposed fp16
copy xT[D, S] so the device never transposes x (keeps PE streaming matmuls).

Per-core math for x [S=8192, D=1024]:
    mu, var per token (LN stats)
    logits = r * (W_cat @ xT - Wsum x mu) + b      (r = rsqrt(var+eps))
    alpha = sigmoid(logits[0:32]); b = logits[32:64]
    h_t = alpha_t * h_{t-1} + b_t                  (scan along seq)
    out = x + [h;1].T @ [W_out.T; b_out]

Device structure per 512-seq superchunk (16 of them), 4-stage pipeline
load(p) | front_a(p-2) | gram-stats(p-1) | back(p-3) | front_b(p-2):
  - load x [128, 4, 1024] and xT-slices yt [128, 8, 512] fp16 (sync HWDGE)
  - front_a: in-proj G[0:65, 512] = W_ext @ yt (row 64 = ones/D -> mu row);
    ACT copies mu row to SBUF
  - stats: token Gram matrices on PE (gr = yt_cb^T @ yt_cb accum over
    d-slices); DVE extracts diag*(1/D) via TT x (I/D) + tensor_reduce -> s2d
  - back: DVE scan -> h; out-proj + residual via identity matmuls into PSUM;
    copies alternate ACT/DVE; store via gpsimd HWDGE
  - front_b: PE transposes mu row->s-layout; var = s2d - mu^2 (DVE);
    r = recip(ACT sqrt(var+eps)) (custom-DVE); PE transposes r -> row;
    rank-1 corr (Wsum_neg x mu_row, one matmul); rb = ones x r per block;
    logits = G * rb (DVE); alpha = ACT sigmoid; bv = DVE bias-add
"""

import sys
from contextlib import ExitStack

if "/opt/trn_rl_repo" not in sys.path:
    sys.path.insert(0, "/opt/trn_rl_repo")

import numpy as np

import concourse.bass as bass
import concourse.bacc as bacc
import concourse.tile as tile
from concourse import mybir
from concourse.bass_utils import run_bass_kernel_spmd

F32 = mybir.dt.float32
F16 = mybir.dt.float16
I32 = mybir.dt.int32
OP = mybir.AluOpType
AF = mybir.ActivationFunctionType

B, S, D = 8, 8192, 1024
HN = 32          # H * n state channels
K2 = 2 * HN      # alpha + b fused projection output channels
KW = K2 + 1      # + ones/D row for the mean
LN_EPS = 1e-5

SC = 512         # seq superchunk
NSC = S // SC    # 16
NB = SC // 128   # 4 seq blocks of 128 per superchunk
ND = D // 128    # 8 d-slices

_PROGRAM_CACHE = {}

CFG = dict(v=6)


def build_program(cfg=CFG, repeat=1):
    nc = bacc.Bacc("TRN2", target_bir_lowering=False, debug=False, num_devices=B)

    x_in = nc.declare_dram_parameter("x", [S, D], F16, isOutput=False)
    xt_in = nc.declare_dram_parameter("xt", [D, S], F16, isOutput=False)
    w_in_d = nc.declare_dram_parameter("w_in", [128, ND, KW], F16, isOutput=False)
    wsn_d = nc.declare_dram_parameter("wsn", [1, K2], F16, isOutput=False)
    ones128_d = nc.declare_dram_parameter("ones128", [128, K2], F16, isOutput=False)
    b_t_d = nc.declare_dram_parameter("b_t", [K2, 1], F32, isOutput=False)
    w_out_d = nc.declare_dram_parameter("w_out", [HN + 1, D], F16, isOutput=False)
    ident_d = nc.declare_dram_parameter("ident", [128, 128], F16, isOutput=False)
    identd_d = nc.declare_dram_parameter("identd", [128, 128], F16, isOutput=False)
    out_d = nc.declare_dram_parameter("out", [S, D], F16, isOutput=True)

    with tile.TileContext(nc) as tc, ExitStack() as ctx:
        consts = ctx.enter_context(tc.tile_pool(name="consts", bufs=1))
        xpool = ctx.enter_context(tc.tile_pool(name="xpool", bufs=5))
        ytpool = ctx.enter_context(tc.tile_pool(name="ytpool", bufs=4))
        stat = ctx.enter_context(tc.tile_pool(name="stat", bufs=3))
        abpool = ctx.enter_context(tc.tile_pool(name="abpool", bufs=3))
        hpool = ctx.enter_context(tc.tile_pool(name="hpool", bufs=3))
        opool = ctx.enter_context(tc.tile_pool(name="opool", bufs=2))
        scr = ctx.enter_context(tc.tile_pool(name="scr", bufs=2))
        psum_g = ctx.enter_context(tc.tile_pool(name="psum_g", bufs=2, space="PSUM"))
        psum_s = ctx.enter_context(tc.tile_pool(name="psum_s", bufs=2, space="PSUM"))
        psum_o = ctx.enter_context(tc.tile_pool(name="psum_o", bufs=2, space="PSUM"))
        psum_gr = ctx.enter_context(tc.tile_pool(name="psum_gr", bufs=1, space="PSUM"))
        psum_mu = ctx.enter_context(tc.tile_pool(name="psum_mu", bufs=1, space="PSUM"))

        # ---- constants ----
        w_in_sb = consts.tile([128, ND, KW], F16)
        nc.sync.dma_start(out=w_in_sb, in_=w_in_d[:, :, :])
        wsn_sb = consts.tile([1, K2], F16)
        nc.sync.dma_start(out=wsn_sb, in_=wsn_d[:, :])
        ones128_sb = consts.tile([128, K2], F16)
        nc.sync.dma_start(out=ones128_sb, in_=ones128_d[:, :])
        b_t_sb = consts.tile([K2, 1], F32)
        nc.sync.dma_start(out=b_t_sb, in_=b_t_d[:, :])
        w_out_sb = consts.tile([HN + 1, D], F16)
        nc.sync.dma_start(out=w_out_sb, in_=w_out_d[:, :])
        ident = consts.tile([128, 128], F16)
        nc.sync.dma_start(out=ident, in_=ident_d[:, :])
        identd = consts.tile([128, 128], F16)
        nc.sync.dma_start(out=identd, in_=identd_d[:, :])
        magic4 = consts.tile([128, NB], I32)
        nc.gpsimd.memset(magic4, 0x5F3759DF)

        def emit_load(sc):
            s0 = sc * SC
            yt = ytpool.tile([128, ND, SC], F16, tag="yt")
            nc.sync.dma_start(
                out=yt,
                in_=xt_in[:, s0 : s0 + SC].rearrange("(j p) s -> p j s", p=128),
            )
            x_t = xpool.tile([128, NB, D], F16, tag="x_t")
            nc.sync.dma_start(
                out=x_t,
                in_=x_in[s0 : s0 + SC, :].rearrange("(c p) d -> p c d", p=128),
            )
            return x_t, yt

        def emit_stats(yt):
            """Per-token sumsq/D via PE Gram diag: s2d [128, NB] f32."""
            s2d = stat.tile([128, NB], F32, tag="s2d")
            gr = psum_gr.tile([128, NB, 128], F32, tag="gr")
            for c in range(NB):
                cb = slice(c * 128, (c + 1) * 128)
                for j in range(ND):
                    nc.tensor.matmul(
                        gr[:, c, :], lhsT=yt[:, j, cb], rhs=yt[:, j, cb],
                        start=(j == 0), stop=(j == ND - 1),
                        skip_group_check=True,
                    )
                junk = scr.tile([128, 128], F32, tag="junk")
                nc.vector.tensor_tensor(
                    out=junk, in0=gr[:, c, :], in1=identd, op=OP.mult
                )
                nc.vector.tensor_reduce(
                    out=s2d[:, c : c + 1], in_=junk,
                    axis=mybir.AxisListType.X, op=OP.add,
                )
            return s2d

        def emit_front_a(sc, yt):
            """In-proj matmuls (only need yt): G[0:65, SC]; row 64 = mu."""
            g_ps = psum_g.tile([128, SC], F32, tag="g")
            for i in range(ND):
                nc.tensor.matmul(
                    g_ps[0:KW, :],
                    lhsT=w_in_sb[:, i, :],
                    rhs=yt[:, i, :],
                    start=(i == 0),
                    stop=False,
                    skip_group_check=True,
                )
            mu_row = stat.tile([1, SC], F16, tag="murow")
            nc.scalar.copy(out=mu_row, in_=g_ps[K2 : K2 + 1, :])
            return g_ps, mu_row

        def emit_front_b(sc, g_ps, mu_row, s2d):
            # mu back to token layout: 4 tiny [1,128] -> [128,1] transposes
            mu_ps = psum_mu.tile([128, NB, 128], F16, tag="mu_ps")
            for c in range(NB):
                nc.tensor.transpose(
                    mu_ps[:, c, 0:1], mu_row[0:1, c * 128 : (c + 1) * 128],
                    ident[0:1, 0:1],
                )
            mu4 = stat.tile([128, NB], F32, tag="mu4")
            nc.vector.tensor_copy(out=mu4, in_=mu_ps[:, :, 0])
            musq = stat.tile([128, NB], F32, tag="musq")
            nc.vector.tensor_tensor(out=musq, in0=mu4, in1=mu4, op=OP.mult)
            me = stat.tile([128, NB], F32, tag="me")
            nc.vector.tensor_scalar(
                out=me, in0=musq, scalar1=-LN_EPS, scalar2=None, op0=OP.add
            )
            var4 = stat.tile([128, NB], F32, tag="v4")
            nc.vector.tensor_tensor(out=var4, in0=s2d, in1=me, op=OP.subtract)
            # rsqrt fully on DVE: bit-trick seed + one Newton step (keeps the
            # ACT LUT pinned on Sigmoid; the Sqrt table was reloading every
            # superchunk, ~1.5us/chunk of ACT_TABLE_LOAD)
            t4 = stat.tile([128, NB], F32, tag="t4")
            r4 = stat.tile([128, NB], F32, tag="r4")
            nc.vector.tensor_scalar(
                out=t4.bitcast(I32), in0=var4.bitcast(I32), scalar1=1,
                scalar2=None, op0=OP.logical_shift_right,
            )
            nc.vector.tensor_tensor(
                out=r4.bitcast(I32), in0=magic4, in1=t4.bitcast(I32),
                op=OP.subtract,
            )
            for _ in range(1):
                nc.vector.tensor_tensor(out=t4, in0=r4, in1=r4, op=OP.mult)
                nc.vector.tensor_tensor(out=t4, in0=t4, in1=var4, op=OP.mult)
                nc.vector.tensor_scalar(
                    out=t4, in0=t4, scalar1=-0.5, scalar2=1.5,
                    op0=OP.mult, op1=OP.add,
                )
                nc.vector.tensor_tensor(out=r4, in0=r4, in1=t4, op=OP.mult)
            # r to row layout: one zero-padded [128, 128] transpose; block
            # c's r lands on row 32*c (partition base 0/32/64/96 - exactly
            # the PE quadrant-legal bases for the 32-row rank-1 reads)
            r_pad = stat.tile([128, NB, HN], F16, tag="rpad")
            nc.gpsimd.memset(r_pad.bitcast(I32), 0)
            nc.vector.tensor_copy(out=r_pad[:, :, 0:1], in_=r4)
            rT_ps = psum_s.tile([HN, NB, 128], F16, tag="rT")
            for c in range(NB):
                nc.tensor.transpose(rT_ps[:, c, :], r_pad[:, c, :], ident)
            rT = stat.tile([HN, NB, 128], F16, tag="rTs")
            nc.scalar.copy(out=rT, in_=rT_ps)

            # mean correction: one rank-1 matmul over the whole superchunk
            # (must follow the ACT mu_row read of g_ps row 64)
            nc.tensor.matmul(
                g_ps[0:K2, :], lhsT=wsn_sb, rhs=mu_row,
                start=False, stop=True, skip_group_check=True,
            )
            # r broadcast: rows 64:128 (recycles the mean row's bank space)
            for c in range(NB):
                nc.tensor.matmul(
                    g_ps[K2 : 2 * K2, c * 128 : (c + 1) * 128],
                    lhsT=ones128_sb[0:HN, :],
                    rhs=rT[:, c, :],
                    start=True, stop=True, skip_group_check=True,
                )
            rb_sb = abpool.tile([K2, SC], F16, tag="rb")
            nc.scalar.copy(out=rb_sb, in_=g_ps[K2 : 2 * K2, :])
            logits = abpool.tile([K2, SC], F16, tag="logits")
            nc.vector.tensor_tensor(
                out=logits, in0=g_ps[0:K2, :], in1=rb_sb, op=OP.mult
            )

            alpha_t = abpool.tile([HN, SC], F16, tag="alpha")
            nc.scalar.activation(
                out=alpha_t, in_=logits[0:HN, :], func=AF.Sigmoid,
                bias=b_t_sb[0:HN], scale=1.0,
            )
            bv_t = abpool.tile([HN, SC], F16, tag="bv")
            nc.vector.tensor_scalar(
                out=bv_t, in0=logits[HN:K2, :],
                scalar1=b_t_sb[HN:K2, 0:1], scalar2=None, op0=OP.add,
            )
            return alpha_t, bv_t

        def emit_back(sc, x_t, alpha_t, bv_t, h_prev):
            s0 = sc * SC
            h_t = hpool.tile([HN + 1, SC], F16, tag="h")
            # two packed fp16 1.0s per int32 (fp16 memset untested on HW)
            nc.gpsimd.memset(h_t[HN : HN + 1, :].bitcast(I32), 0x3C003C00)
            nc.vector.tensor_tensor_scan(
                out=h_t[0:HN, :],
                data0=alpha_t,
                data1=bv_t,
                initial=0.0 if h_prev is None else h_prev[0:HN, SC - 1 : SC],
                op0=OP.mult,
                op1=OP.add,
            )

            o_sb = opool.tile([128, NB, D], F16, tag="o_sb")
            for c in range(NB):
                lhs = h_t[:, c * 128 : (c + 1) * 128]
                for half in range(2):
                    hs = slice(half * 512, (half + 1) * 512)
                    o_ps = psum_o.tile([128, 512], F32, tag="ops")
                    if (c * 2 + half) % 2 == 0:
                        # ACT can't add: residual via identity matmul in PSUM
                        nc.tensor.matmul(
                            o_ps, lhsT=lhs, rhs=w_out_sb[:, hs],
                            start=True, stop=False, skip_group_check=True,
                        )
                        nc.tensor.matmul(
                            o_ps, lhsT=ident, rhs=x_t[:, c, hs],
                            start=False, stop=True, skip_group_check=True,
                        )
                        nc.scalar.copy(out=o_sb[:, c, hs], in_=o_ps)
                    else:
                        # DVE fuses the residual into the PSUM drain
                        nc.tensor.matmul(
                            o_ps, lhsT=lhs, rhs=w_out_sb[:, hs],
                            start=True, stop=True, skip_group_check=True,
                        )
                        nc.vector.tensor_tensor(
                            out=o_sb[:, c, hs], in0=o_ps, in1=x_t[:, c, hs],
                            op=OP.add,
                        )
            nc.gpsimd.dma_start(
                out=out_d[s0 : s0 + SC, :].rearrange("(c p) d -> p c d", p=128),
                in_=o_sb,
            )
            return h_t

        for _rep in range(repeat):
            # 5-slot pipeline, emission order chosen so the PE's in-order
            # stream always has ready work between cross-engine round-trips:
            # front_a(p-2) | stats(p-1) | back(p-3) | front_b(p-2)
            xs, rs, fa, fr = {}, {}, {}, {}
            h_prev = None
            for p in range(NSC + 3):
                if p < NSC:
                    xs[p] = emit_load(p)
                if 2 <= p <= NSC + 1:
                    sc = p - 2
                    fa[sc] = emit_front_a(sc, xs[sc][1])
                if 1 <= p <= NSC:
                    rs[p - 1] = emit_stats(xs[p - 1][1])
                if p >= 3:
                    sc = p - 3
                    alpha_t, bv_t = fr.pop(sc)
                    h_prev = emit_back(sc, xs[sc][0], alpha_t, bv_t, h_prev)
                    del xs[sc]
                if 2 <= p <= NSC + 1:
                    sc = p - 2
                    g_ps, mu_row = fa.pop(sc)
                    fr[sc] = emit_front_b(sc, g_ps, mu_row, rs.pop(sc))

    nc.compile()
    return nc


def _prep_host_inputs(x, W_a, b_a, W_in, b_in, W_out, b_out, ln_gamma, ln_beta):
    f = np.float32
    W_cat = np.concatenate(
        [W_a * ln_gamma[None, :], W_in * ln_gamma[None, :]], axis=0
    ).astype(f)  # [64, 1024]
    # 65th row: ones/D (mean of RAW x; not gamma-folded)
    W_ext = np.concatenate(
        [W_cat, np.full((1, D), 1.0 / D, dtype=f)], axis=0
    )  # [65, 1024]
    w_in_host = (
        np.ascontiguousarray(W_ext.T.reshape(ND, 128, KW).transpose(1, 0, 2))
        .astype(np.float16)
    )  # [128, 8, 65]
    wsn_host = (-W_cat.sum(axis=1))[None, :].astype(np.float16)  # [1, 64]
    ones128_host = np.ones((128, K2), dtype=np.float16)
    b_t_host = np.concatenate(
        [b_a + W_a @ ln_beta, b_in + W_in @ ln_beta], axis=0
    ).astype(f)[:, None]  # [64, 1]
    w_out_host = (
        np.ascontiguousarray(np.concatenate([W_out.T, b_out[None, :]], axis=0))
        .astype(np.float16)
    )  # [33, 1024]
    ident_host = np.eye(128, dtype=np.float16)
    identd_host = (np.eye(128) / 1024.0).astype(np.float16)
    shared = {
        "w_in": w_in_host,
        "wsn": wsn_host,
        "ones128": ones128_host,
        "b_t": b_t_host,
        "w_out": w_out_host,
        "ident": ident_host,
        "identd": identd_host,
    }
    in_maps = []
    for i in range(B):
        xi = np.ascontiguousarray(x[i]).astype(np.float16)
        m = {"x": xi, "xt": np.ascontiguousarray(xi.T), **shared}
        in_maps.append(m)
    return in_maps


def run(inputs, trace=False, cfg=CFG):
    key = str(sorted(cfg.items()))
    if key not in _PROGRAM_CACHE:
        _PROGRAM_CACHE[key] = build_program(cfg)
    nc = _PROGRAM_CACHE[key]
    in_maps = _prep_host_inputs(**inputs)
    res = run_bass_kernel_spmd(nc, in_maps, list(range(B)), trace=trace)
    out = np.stack(
        [res.results[i]["out"].astype(np.float32) for i in range(B)], axis=0
    )
    return out, res


def kernel(**inputs):
    out, _ = run(inputs)
    return out
